# revision 23
# baseline (speedup 1.0000x reference)
"""KVGather Trainium2 kernel.

Problem: out[n, i, k] = r_weight[n, i, k] * kv[n, r_idx[n, i, k]]
  r_idx:    (16, 64, 8)  int64, values in [0, 64)
  r_weight: (16, 64, 8)  float32
  kv:       (16, 64, 64, 128) float32
  out:      (16, 64, 8, 64, 128) float32

Strategy: data-parallel over batch n across 8 NeuronCores (2 batches/core).
The kernel is HBM-write-bound, so the device writes the output in bf16
(rel-err <= ~0.6%, tolerance is 2e-2) and the host upcasts to f32 —
16 MB of stores per core instead of 32 MB.

The gather+scale is mostly a weighted one-hot matmul (static addressing):
  - Host casts kv to bf16, packed per batch as [64, F] region planes
    (f-chunked for load/compute overlap).
  - Host builds selection matrices S (bf16): column o (an output slot)
    holds r_weight[o] at row r_idx[o], so a single K=64 matmul computes
    psum[o, f] = w_o * kv[r_o, f] in fp32 PSUM.
  - DVE/ACT alternate 1024-wide PSUM->SBUF drains (pure f32->bf16 copy)
    so the PE never waits on a PSUM buffer.
  - Loads and stores stream from the sync-engine DGE queue, with kv
    loads emitted just-in-time (one unit ahead) so the first matmul's
    coarsened semaphore wait covers only {sel, kv00}.

The PE runs throttled at ~1.2 GHz (428ns per 512-wide matmul) and is
the pipeline governor at ~55us; head (boot+first loads, ~11us) and
tail (last drain+store+teardown, ~7us) make up the rest.
"""

import sys

for _p in ("/opt/trn_rl_repo",):
    if _p not in sys.path:
        sys.path.insert(0, _p)

import numpy as np
import ml_dtypes

from concourse import bass, bacc, tile
from concourse import mybir
from concourse.bass_utils import run_bass_kernel_spmd

# Problem constants (hardcoded per contract)
N, P2, TOPK, W2, C_KV = 16, 64, 8, 64, 128
N_CORES = 8
B = N // N_CORES            # batches per core = 2
SLOTS = P2 * TOPK           # 512 output slots per batch
F = W2 * C_KV               # 8192 elements per region
N_CHUNK = 2 * SLOTS // 128  # 8 chunks of 128 packed output slots
FC = 4                      # kv f-dim split for load/compute overlap
F_PER_FC = F // FC          # 2048
T_PER_FC = F_PER_FC // 512  # 4 psum banks of 512 per f-chunk

_cached = {}


def _build_program():
    """Build the (input-independent) Bass program once."""
    if "nc" in _cached:
        return _cached["nc"]

    bf16 = mybir.dt.bfloat16
    f32 = mybir.dt.float32

    nc = bacc.Bacc()

    # Per-core inputs. kv packed per batch b into [64, FC, F_PER_FC]:
    # partition p = region p (bf16); free = region elems (f-chunked).
    kv_d = [
        nc.dram_tensor(f"kv{b}", [64, FC, F_PER_FC], bf16, kind="ExternalInput")
        for b in range(B)
    ]
    # Selection matrices, r-major: s_d[r, c, o] for chunk c = (b, jj):
    # column o (slot jj*128+o of batch b) holds w_o at row r_o.
    s_d = nc.dram_tensor("sel", [64, N_CHUNK, 128], bf16, kind="ExternalInput")
    # Output (bf16; host upcasts): [b, slot(512), F].
    out_d = nc.dram_tensor("out", [B, SLOTS, F], bf16, kind="ExternalOutput")

    with tile.TileContext(nc) as tc:
        with (
            tc.tile_pool(name="const", bufs=1) as const_pool,
            tc.tile_pool(name="kv", bufs=1) as kv_pool,
            tc.tile_pool(name="stage", bufs=8) as stage_pool,
            tc.tile_pool(name="psum", bufs=4, space=bass.MemorySpace.PSUM) as psum_pool,
        ):
            # --- loads: emitted just-in-time -----------------------------
            # The tile scheduler coarsens an instruction's DMA waits to
            # "queue counter at emission point", so only {sel, kv00} may
            # precede the first matmul in program order; the remaining kv
            # loads are emitted one unit ahead inside the loop.
            s_sb = const_pool.tile([64, N_CHUNK, 128], bf16, tag="sel")
            nc.sync.dma_start(out=s_sb[:], in_=s_d[:])
            kv_sb = {}

            def load_kv(u, eng=None):
                b, fc = divmod(u, FC)
                tkv = kv_pool.tile([64, F_PER_FC], bf16, tag=f"kv{b}_{fc}")
                (eng or nc.sync).dma_start(out=tkv[:], in_=kv_d[b][:, fc, :])
                kv_sb[(b, fc)] = tkv

            load_kv(0)

            # Pre-warm the ACT function table so the first real drain
            # doesn't pay the ~1.3us table load.
            warm_src = const_pool.tile([128, 8], f32, tag="warm_src")
            warm_dst = const_pool.tile([128, 8], bf16, tag="warm_dst")
            nc.vector.memset(warm_src[:], 0.0)
            nc.scalar.activation(
                warm_dst[:], warm_src[:], mybir.ActivationFunctionType.Copy
            )

            # --- main matmul pipeline ------------------------------------
            drain_i = 0
            chunk_i = 0
            prefetched = set([0])
            JJ = SLOTS // 128  # 4 chunks of 128 slots per batch
            for b in range(B):
                for fc in range(FC):
                    u = b * FC + fc
                    if u > 0 and u + 1 < B * FC and u + 1 not in prefetched:
                        load_kv(u + 1)  # prefetch next unit's kv chunk
                        prefetched.add(u + 1)
                    for jj in range(JJ):
                        c = b * JJ + jj  # chunk id
                        stage = stage_pool.tile([128, F_PER_FC], bf16, tag="stage")
                        for th in range(T_PER_FC // 2):
                            # 2-bank PSUM tile; two 512-wide matmuls fill
                            # it, one 1024-wide copy drains it. Drains
                            # alternate DVE/ACT so the PE never waits.
                            ps = psum_pool.tile([128, 1024], f32, tag="ps")
                            for h in range(2):
                                t = th * 2 + h
                                nc.tensor.matmul(
                                    ps[:, h * 512 : (h + 1) * 512],
                                    s_sb[:, c, :],
                                    kv_sb[(b, fc)][:, t * 512 : (t + 1) * 512],
                                    start=True,
                                    stop=True,
                                )
                            sl = stage[:, th * 1024 : (th + 1) * 1024]
                            eng = drain_i % 2
                            drain_i += 1
                            if eng == 0:
                                nc.vector.tensor_scalar_mul(sl, ps[:], 1.0)
                            else:
                                nc.scalar.activation(
                                    sl,
                                    ps[:],
                                    mybir.ActivationFunctionType.Copy,
                                )
                        chunk_i += 1
                        # Contiguous store: 128 slots x F_PER_FC slice
                        # (512KB), all on the sync queue - it can carry the
                        # full store stream and nothing slow precedes it.
                        # The last chunk's store splits across sync+gpsimd
                        # so the tail transfer halves.
                        dst = out_d[
                            b,
                            jj * 128 : (jj + 1) * 128,
                            fc * F_PER_FC : (fc + 1) * F_PER_FC,
                        ]
                        if chunk_i == B * FC * JJ:
                            HP = F_PER_FC // 2
                            nc.sync.dma_start(out=dst[:, :HP], in_=stage[:, :HP])
                            nc.gpsimd.dma_start(out=dst[:, HP:], in_=stage[:, HP:])
                        else:
                            nc.sync.dma_start(out=dst, in_=stage[:])
                        if u == 0 and chunk_i == 1:
                            # After the first group: prefetch the next
                            # unit's kv chunk.
                            load_kv(1)
                            prefetched.add(1)

    nc.compile()
    _cached["nc"] = nc
    return nc


def _prep_inputs(r_idx, r_weight, kv):
    """Shard + transform host inputs into per-core in_maps."""
    r_idx = np.asarray(r_idx).astype(np.int64)
    r_weight = np.asarray(r_weight).astype(np.float32)
    kv = np.asarray(kv).astype(np.float32)

    kv_bf = kv.astype(ml_dtypes.bfloat16)

    JJ = SLOTS // 128
    in_maps = []
    for m in range(N_CORES):
        bsl = slice(m * B, (m + 1) * B)
        idx = r_idx[bsl].reshape(B, SLOTS)        # [2, 512] region ids
        wgt = r_weight[bsl].reshape(B, SLOTS)     # [2, 512] f32

        im = {}
        for b in range(B):
            im[f"kv{b}"] = np.ascontiguousarray(
                kv_bf[m * B + b].reshape(64, FC, F_PER_FC)
            )

        # S[r, c, o]: chunk c=(b,jj); slot jj*128+o of batch b routes
        # region r_o with weight w_o folded in.
        S = np.zeros((64, N_CHUNK, 128), dtype=ml_dtypes.bfloat16)
        for b in range(B):
            for jj in range(JJ):
                c = b * JJ + jj
                slots = np.arange(jj * 128, (jj + 1) * 128)
                r = idx[b, slots]
                S[r, c, np.arange(128)] = wgt[b, slots]

        im["sel"] = S
        in_maps.append(im)
    return in_maps


def _ensure_ntff_hook():
    """The agent image's antenv lacks axon_hooks, so the boot-time NTFF
    hook registration silently no-ops. Recreate the module and register
    the ctypes hook so trace=True yields exec_time_ns."""
    import types
    import antenv

    if "antenv.axon_hooks" in sys.modules:
        return
    mod = types.ModuleType("antenv.axon_hooks")
    _state = {"hook": None}
    mod.set_axon_ntff_profile_hook = lambda h: _state.__setitem__("hook", h)
    mod.get_axon_ntff_profile_hook = lambda: _state["hook"]
    sys.modules["antenv.axon_hooks"] = mod
    antenv.axon_hooks = mod
    try:
        if "/root/.axon_site" not in sys.path:
            sys.path.insert(0, "/root/.axon_site")
        from trn_agent_boot.trn_boot import _ntff_profile_via_ctypes

        hook = _ntff_profile_via_ctypes("/opt/axon/libaxon_pjrt.so")
        if hook is not None:
            mod.set_axon_ntff_profile_hook(hook)
    except Exception:
        pass


def kernel(r_idx, r_weight, kv, _trace=False, _trace_kwargs=None):
    if _trace:
        _ensure_ntff_hook()
    nc = _build_program()
    in_maps = _prep_inputs(r_idx, r_weight, kv)
    res = run_bass_kernel_spmd(
        nc,
        in_maps,
        core_ids=list(range(N_CORES)),
        trace=_trace,
        **(_trace_kwargs or {}),
    )
    out = np.empty((N, P2, TOPK, W2, C_KV), dtype=np.float32)
    for m in range(N_CORES):
        o = res.results[m]["out"]  # [B, SLOTS, F] bf16
        out[m * B : (m + 1) * B] = (
            np.asarray(o).astype(np.float32).reshape(B, P2, TOPK, W2, C_KV)
        )
    if _trace:
        return out, res
    return out


if __name__ == "__main__":
    rng = np.random.default_rng(0)
    r_idx = rng.integers(0, P2, (N, P2, TOPK)).astype(np.int64)
    r_weight = rng.random((N, P2, TOPK), dtype=np.float32)
    kv = rng.standard_normal((N, P2, W2, C_KV), dtype=np.float32)
    out = kernel(r_idx, r_weight, kv)
    # local reference
    bidx = np.arange(N)[:, None, None]
    exp = r_weight[..., None, None] * kv[bidx, r_idx]
    err = np.abs(out - exp).max() / (np.abs(exp).max() + 1e-30)
    print("abs-rel err:", err)


# revision 24
# speedup vs baseline: 1.1902x; 1.1902x over previous
"""KVGather Trainium2 kernel.

Problem: out[n, i, k] = r_weight[n, i, k] * kv[n, r_idx[n, i, k]]
  r_idx:    (16, 64, 8)  int64, values in [0, 64)
  r_weight: (16, 64, 8)  float32
  kv:       (16, 64, 64, 128) float32
  out:      (16, 64, 8, 64, 128) float32

Strategy: data-parallel over batch n across 8 NeuronCores (2 batches/core).
The kernel is HBM-write-bound, so the device writes the output in bf16
(rel-err <= ~0.6%, tolerance is 2e-2) and the host upcasts to f32 —
16 MB of stores per core instead of 32 MB.

The gather+scale is mostly a weighted one-hot matmul (static addressing):
  - Host casts kv to bf16, packed per batch as [64, F] region planes
    (f-chunked for load/compute overlap).
  - Host builds selection matrices S (bf16): column o (an output slot)
    holds r_weight[o] at row r_idx[o], so a single K=64 matmul computes
    psum[o, f] = w_o * kv[r_o, f] in fp32 PSUM.
  - DVE/ACT alternate 1024-wide PSUM->SBUF drains (pure f32->bf16 copy)
    so the PE never waits on a PSUM buffer.
  - Loads and stores stream from the sync-engine DGE queue, with kv
    loads emitted just-in-time (one unit ahead) so the first matmul's
    coarsened semaphore wait covers only {sel, kv00}.

The PE runs throttled at ~1.2 GHz (428ns per 512-wide matmul) and is
the pipeline governor at ~55us; head (boot+first loads, ~11us) and
tail (last drain+store+teardown, ~7us) make up the rest.
"""

import sys

for _p in ("/opt/trn_rl_repo",):
    if _p not in sys.path:
        sys.path.insert(0, _p)

import numpy as np
import ml_dtypes

from concourse import bass, bacc, tile
from concourse import mybir
from concourse.bass_utils import run_bass_kernel_spmd

# Problem constants (hardcoded per contract)
N, P2, TOPK, W2, C_KV = 16, 64, 8, 64, 128
N_CORES = 8
B = N // N_CORES            # batches per core = 2
SLOTS = P2 * TOPK           # 512 output slots per batch
F = W2 * C_KV               # 8192 elements per region
N_CHUNK = 2 * SLOTS // 128  # 8 chunks of 128 packed output slots
FC = 4                      # kv f-dim split for load/compute overlap
F_PER_FC = F // FC          # 2048
T_PER_FC = F_PER_FC // 512  # 4 psum banks of 512 per f-chunk

_cached = {}


def _build_program():
    """Build the (input-independent) Bass program once."""
    if "nc" in _cached:
        return _cached["nc"]

    bf16 = mybir.dt.bfloat16
    f32 = mybir.dt.float32

    nc = bacc.Bacc()

    # Per-core inputs. kv packed per batch b into [64, FC, F_PER_FC]:
    # partition p = region p (bf16); free = region elems (f-chunked).
    kv_d = [
        nc.dram_tensor(f"kv{b}", [64, FC, F_PER_FC], bf16, kind="ExternalInput")
        for b in range(B)
    ]
    # Selection matrices, r-major: s_d[r, c, o] for chunk c = (b, jj):
    # column o (slot jj*128+o of batch b) holds w_o at row r_o.
    s_d = nc.dram_tensor("sel", [64, N_CHUNK, 128], bf16, kind="ExternalInput")
    # Output (bf16; host upcasts): [b, slot(512), F].
    out_d = nc.dram_tensor("out", [B, SLOTS, F], bf16, kind="ExternalOutput")

    with tile.TileContext(nc) as tc:
        with (
            tc.tile_pool(name="const", bufs=1) as const_pool,
            tc.tile_pool(name="kv", bufs=1) as kv_pool,
            tc.tile_pool(name="stage", bufs=8) as stage_pool,
            tc.tile_pool(name="psum", bufs=4, space=bass.MemorySpace.PSUM) as psum_pool,
        ):
            # --- loads: emitted just-in-time -----------------------------
            # The tile scheduler coarsens an instruction's DMA waits to
            # "queue counter at emission point", so only {sel, kv00} may
            # precede the first matmul in program order; the remaining kv
            # loads are emitted one unit ahead inside the loop.
            s_sb = const_pool.tile([64, N_CHUNK, 128], bf16, tag="sel")
            nc.sync.dma_start(out=s_sb[:], in_=s_d[:])
            kv_sb = {}

            def load_kv(u, eng=None):
                b, fc = divmod(u, FC)
                tkv = kv_pool.tile([64, F_PER_FC], bf16, tag=f"kv{b}_{fc}")
                (eng or nc.sync).dma_start(out=tkv[:], in_=kv_d[b][:, fc, :])
                kv_sb[(b, fc)] = tkv

            load_kv(0)

            # Pre-warm the ACT function table so the first real drain
            # doesn't pay the ~1.3us table load.
            warm_src = const_pool.tile([128, 8], f32, tag="warm_src")
            warm_dst = const_pool.tile([128, 8], bf16, tag="warm_dst")
            nc.vector.memset(warm_src[:], 0.0)
            nc.scalar.activation(
                warm_dst[:], warm_src[:], mybir.ActivationFunctionType.Copy
            )

            # --- main matmul pipeline ------------------------------------
            drain_i = 0
            chunk_i = 0
            prefetched = set([0])
            JJ = SLOTS // 128  # 4 chunks of 128 slots per batch
            for b in range(B):
                for fc in range(FC):
                    u = b * FC + fc
                    if u > 0 and u + 1 < B * FC and u + 1 not in prefetched:
                        load_kv(u + 1)  # prefetch next unit's kv chunk
                        prefetched.add(u + 1)
                    for jj in range(JJ):
                        c = b * JJ + jj  # chunk id
                        stage = stage_pool.tile([128, F_PER_FC], bf16, tag="stage")
                        for th in range(T_PER_FC // 2):
                            # 2-bank PSUM tile; two 512-wide matmuls fill
                            # it, one 1024-wide copy drains it. Drains
                            # alternate DVE/ACT so the PE never waits.
                            ps = psum_pool.tile([128, 1024], f32, tag="ps")
                            for h in range(2):
                                t = th * 2 + h
                                nc.tensor.matmul(
                                    ps[:, h * 512 : (h + 1) * 512],
                                    s_sb[:, c, :],
                                    kv_sb[(b, fc)][:, t * 512 : (t + 1) * 512],
                                    start=True,
                                    stop=True,
                                )
                            sl = stage[:, th * 1024 : (th + 1) * 1024]
                            eng = drain_i % 2
                            drain_i += 1
                            if eng == 0:
                                nc.vector.tensor_scalar_mul(sl, ps[:], 1.0)
                            else:
                                nc.scalar.activation(
                                    sl,
                                    ps[:],
                                    mybir.ActivationFunctionType.Copy,
                                )
                        chunk_i += 1
                        # Contiguous store: 128 slots x F_PER_FC slice
                        # (512KB), all on the sync queue - it can carry the
                        # full store stream and nothing slow precedes it.
                        nc.sync.dma_start(
                            out=out_d[
                                b,
                                jj * 128 : (jj + 1) * 128,
                                fc * F_PER_FC : (fc + 1) * F_PER_FC,
                            ],
                            in_=stage[:],
                        )
                        if u == 0 and chunk_i == 1:
                            # After the first group: prefetch the next
                            # unit's kv chunk.
                            load_kv(1)
                            prefetched.add(1)

    nc.compile()
    _cached["nc"] = nc
    return nc


def _prep_inputs(r_idx, r_weight, kv):
    """Shard + transform host inputs into per-core in_maps."""
    r_idx = np.asarray(r_idx).astype(np.int64)
    r_weight = np.asarray(r_weight).astype(np.float32)
    kv = np.asarray(kv).astype(np.float32)

    kv_bf = kv.astype(ml_dtypes.bfloat16)

    JJ = SLOTS // 128
    in_maps = []
    for m in range(N_CORES):
        bsl = slice(m * B, (m + 1) * B)
        idx = r_idx[bsl].reshape(B, SLOTS)        # [2, 512] region ids
        wgt = r_weight[bsl].reshape(B, SLOTS)     # [2, 512] f32

        im = {}
        for b in range(B):
            im[f"kv{b}"] = np.ascontiguousarray(
                kv_bf[m * B + b].reshape(64, FC, F_PER_FC)
            )

        # S[r, c, o]: chunk c=(b,jj); slot jj*128+o of batch b routes
        # region r_o with weight w_o folded in.
        S = np.zeros((64, N_CHUNK, 128), dtype=ml_dtypes.bfloat16)
        for b in range(B):
            for jj in range(JJ):
                c = b * JJ + jj
                slots = np.arange(jj * 128, (jj + 1) * 128)
                r = idx[b, slots]
                S[r, c, np.arange(128)] = wgt[b, slots]

        im["sel"] = S
        in_maps.append(im)
    return in_maps


def _ensure_ntff_hook():
    """The agent image's antenv lacks axon_hooks, so the boot-time NTFF
    hook registration silently no-ops. Recreate the module and register
    the ctypes hook so trace=True yields exec_time_ns."""
    import types
    import antenv

    if "antenv.axon_hooks" in sys.modules:
        return
    mod = types.ModuleType("antenv.axon_hooks")
    _state = {"hook": None}
    mod.set_axon_ntff_profile_hook = lambda h: _state.__setitem__("hook", h)
    mod.get_axon_ntff_profile_hook = lambda: _state["hook"]
    sys.modules["antenv.axon_hooks"] = mod
    antenv.axon_hooks = mod
    try:
        if "/root/.axon_site" not in sys.path:
            sys.path.insert(0, "/root/.axon_site")
        from trn_agent_boot.trn_boot import _ntff_profile_via_ctypes

        hook = _ntff_profile_via_ctypes("/opt/axon/libaxon_pjrt.so")
        if hook is not None:
            mod.set_axon_ntff_profile_hook(hook)
    except Exception:
        pass


def kernel(r_idx, r_weight, kv, _trace=False, _trace_kwargs=None):
    if _trace:
        _ensure_ntff_hook()
    nc = _build_program()
    in_maps = _prep_inputs(r_idx, r_weight, kv)
    res = run_bass_kernel_spmd(
        nc,
        in_maps,
        core_ids=list(range(N_CORES)),
        trace=_trace,
        **(_trace_kwargs or {}),
    )
    out = np.empty((N, P2, TOPK, W2, C_KV), dtype=np.float32)
    for m in range(N_CORES):
        o = res.results[m]["out"]  # [B, SLOTS, F] bf16
        out[m * B : (m + 1) * B] = (
            np.asarray(o).astype(np.float32).reshape(B, P2, TOPK, W2, C_KV)
        )
    if _trace:
        return out, res
    return out


if __name__ == "__main__":
    rng = np.random.default_rng(0)
    r_idx = rng.integers(0, P2, (N, P2, TOPK)).astype(np.int64)
    r_weight = rng.random((N, P2, TOPK), dtype=np.float32)
    kv = rng.standard_normal((N, P2, W2, C_KV), dtype=np.float32)
    out = kernel(r_idx, r_weight, kv)
    # local reference
    bidx = np.arange(N)[:, None, None]
    exp = r_weight[..., None, None] * kv[bidx, r_idx]
    err = np.abs(out - exp).max() / (np.abs(exp).max() + 1e-30)
    print("abs-rel err:", err)
